# revision 10
# baseline (speedup 1.0000x reference)
"""Trainium2 Bass kernel for nn_Attention_69801808495308.

Softmax-free attention: attn = cos_w*cossim + cov_w*cov/d + var_w*varprod/d is
linear in k-side summaries, so attn @ f_v reassociates into per-head 64x64
matrices (linear-attention trick) - no NxN score matrix is materialized.

Per (group g, head h), with fk/fv/fq the TRUE projected features (the
1/sigma LayerNorm scale is folded into each PSUM evacuation):
  M1 = (fk/||fk||)^T fv        [64,64]
  M2 = (fk - mean(fk))^T fv    [64,64]   (columns sum to 0 -> q-centering free)
  m3 = kvar^T fv               [64]
  out = [cos_w*(fq/||fq||)@M1 + (cov_w/d)*fq@M2] @ woT + qvar @ RW,
        RW = (var_w/d)*blockdiag(m3) @ woT

Sharding: 8 cores = (group g) x (row half s); one pair-wise fp16 AllReduce
(~140KB) of [B; RW] is the only cross-core communication, issued before the
entire q-side pipeline so its latency is hidden.

v3 engine balance (trace-driven):
- two HWDGE rings: ACT ring carries the 6 big fp16 x loads + uqT transposes,
  SP ring carries weights, v-slab transposes, collective I/O and out writes.
- PE warm-up burst keeps the HAM clock-gate at 8/8 from the first real MM.
- fq/fk evacuations write scaled TRUE values straight into the U-tensor
  component-1 slots; per-head means on GpSimd pool; squares/sums split
  between DVE and ACT; per-tile kvar/qvar chains; M-chain + m3 run two
  tiles behind the projections so PE never waits on DVE.
- attention matmuls batched per (head, 4-tile quad); fp16 output.
"""
import numpy as np
from contextlib import ExitStack

import concourse.bass as bass
from concourse import bacc
import concourse.tile as tile
import concourse.mybir as mybir
from concourse.bass_utils import run_bass_kernel_spmd
from concourse.masks import make_identity

f32 = mybir.dt.float32
fp16 = mybir.dt.float16
ALU = mybir.AluOpType
ACTF = mybir.ActivationFunctionType
AXX = mybir.AxisListType.X
POOL_AVG = mybir.PoolFunctionType.avg

QG, N, D = 4, 2048, 512
H, HD = 8, 64
P = 128
LN_EPS = 1e-5
TQ, TK = N // 2, N // 2
QT, KT = TQ // P, TK // P
NCORES = 8


def build_kernel(cos_w, cov_w, var_w):
    c_cov = cov_w / HD
    c_var = var_w / HD

    nc = bacc.Bacc("TRN2", target_bir_lowering=False, debug=False,
                   num_devices=NCORES)
    xq_d = nc.declare_dram_parameter("xq", [TQ, D], fp16, isOutput=False)
    xk_d = nc.declare_dram_parameter("xk", [TK, D], fp16, isOutput=False)
    xv_d = nc.declare_dram_parameter("xv", [TK, D], fp16, isOutput=False)
    wgT_d = nc.declare_dram_parameter("wgT", [D, D], fp16, isOutput=False)
    woT_d = nc.declare_dram_parameter("woT", [D, D], fp16, isOutput=False)
    out_d = nc.declare_dram_parameter("out", [TQ, D], fp16, isOutput=True)

    with tile.TileContext(nc) as tc, ExitStack() as ctx:
        cp = ctx.enter_context(tc.tile_pool(name="cp", bufs=1))
        slp = ctx.enter_context(tc.tile_pool(name="slp", bufs=4))
        sp = ctx.enter_context(tc.tile_pool(name="sp", bufs=8))
        evp = ctx.enter_context(tc.tile_pool(name="evp", bufs=3))

        ident16 = cp.tile([P, P], fp16)
        eps_b = cp.tile([P, 1], f32)
        nc.vector.memset(eps_b[:], LN_EPS)

        xk_all = cp.tile([P, KT, D], fp16)
        xv_all = cp.tile([P, KT, D], fp16)
        xq_all = cp.tile([P, QT, D], fp16)
        wgT_sb = cp.tile([P, 4, D], fp16)
        woT_sb = cp.tile([P, 4, D], fp16)
        fv_all = cp.tile([P, KT, D], fp16)
        uk_all = cp.tile([P, KT, H, 2, HD], fp16)   # [.,.,.,0]=khat [.,.,.,1]=fk
        uq_all = cp.tile([P, QT, H, 2, HD], fp16)   # [.,.,.,0]=qhat [.,.,.,1]=fq
        uqT_all = cp.tile([P, QT, H, P], fp16)
        st2_k = cp.tile([P, KT, 2], f32)
        st2_v = cp.tile([P, KT, 2], f32)
        st2_q = cp.tile([P, QT, 2], f32)
        kvcol = cp.tile([P, KT, H], fp16)
        qv_all = cp.tile([P, QT, H], fp16)
        qvT_all = cp.tile([H, QT, P], fp16)

        # ---- big fp16 loads on the ACT HWDGE ring ----
        nc.scalar.dma_start(
            xk_all[:, 0:KT // 2, :],
            xk_d[0:TK // 2, :].rearrange("(t p) d -> p t d", p=P))
        nc.scalar.dma_start(
            xv_all[:, 0:KT // 2, :],
            xv_d[0:TK // 2, :].rearrange("(t p) d -> p t d", p=P))
        nc.scalar.dma_start(
            xk_all[:, KT // 2:KT, :],
            xk_d[TK // 2:TK, :].rearrange("(t p) d -> p t d", p=P))
        nc.scalar.dma_start(
            xv_all[:, KT // 2:KT, :],
            xv_d[TK // 2:TK, :].rearrange("(t p) d -> p t d", p=P))
        nc.scalar.dma_start(
            xq_all[:, 0:QT // 2, :],
            xq_d[0:TQ // 2, :].rearrange("(t p) d -> p t d", p=P))
        nc.scalar.dma_start(
            xq_all[:, QT // 2:QT, :],
            xq_d[TQ // 2:TQ, :].rearrange("(t p) d -> p t d", p=P))
        # ---- weights on the SP ring ----
        nc.sync.dma_start(wgT_sb[:], wgT_d[:].rearrange("(c p) n -> p c n", p=P))
        nc.sync.dma_start(woT_sb[:], woT_d[:].rearrange("(c p) n -> p c n", p=P))

        make_identity(nc, ident16)
        bdmask = cp.tile([H, 512], f32)
        nc.gpsimd.memset(bdmask[:], 0.0)
        nc.gpsimd.affine_select(
            out=bdmask[:].rearrange("p (b d) -> p b d", b=H),
            in_=bdmask[:].rearrange("p (b d) -> p b d", b=H),
            compare_op=ALU.not_equal, fill=1.0, base=0,
            pattern=[[-1, H], [0, HD]], channel_multiplier=1)

        psF = ctx.enter_context(tc.tile_pool(name="psF", bufs=3, space="PSUM"))
        psT = ctx.enter_context(tc.tile_pool(name="psT", bufs=2, space="PSUM"))
        psM = ctx.enter_context(tc.tile_pool(name="psM", bufs=1, space="PSUM"))
        psA = ctx.enter_context(tc.tile_pool(name="psA", bufs=2, space="PSUM"))

        # ---- PE warm-up: HAM reaches 8/8 while the first loads land ----
        ps_w = psF.tile([P, D], f32, tag="pf", name="warm")
        for i in range(40):
            nc.tensor.matmul(ps_w[:, 0:P], ident16[:], ident16[:],
                             start=True, stop=True)

        def stage1(x_all, t, st2_all):
            xt = x_all[:, t, :]
            st6 = sp.tile([P, 6], f32, tag="st6")
            nc.vector.bn_stats(st6[:], xt)
            nc.vector.bn_aggr(st2_all[:, t, :], st6[:])
            nc.scalar.activation(xt, xt, ACTF.Identity,
                                 bias=st2_all[:, t, 0:1], scale=-1.0)
            return xt

        def stage2(xt, dst, pe_transpose, evac_scale):
            """Transpose (PE or DMA) + 4-matmul projection + scaled ACT evac
            into dst (the TRUE feature values land there)."""
            slab = slp.tile([P, 4, P], fp16, tag="slab")
            if pe_transpose:
                for c in range(4):
                    pt = psT.tile([P, P], fp16, tag="ptx")
                    nc.tensor.transpose(pt[:], xt[:, c * P:(c + 1) * P], ident16[:])
                    if c % 2 == 0:
                        nc.scalar.copy(slab[:, c, :], pt[:])
                    else:
                        nc.vector.tensor_copy(slab[:, c, :], pt[:])
            else:
                nc.sync.dma_start_transpose(slab[:], xt)
            psf = psF.tile([P, D], f32, tag="pf")
            for c in range(4):
                nc.tensor.matmul(psf[:], slab[:, c, :], wgT_sb[:, c, :],
                                 start=(c == 0), stop=(c == 3))
            nc.scalar.activation(dst, psf[:], ACTF.Copy, scale=evac_scale)

        def head_stats(comp1, sum_dst, sq_dst, kvar_dst, uk0_dst):
            """comp1 holds TRUE f [P,H,64]. Per-head sum + sum-sq on DVE,
            khat & kvar derivations; center comp1 in place."""
            with nc.allow_low_precision(reason="head sums fit fp16"):
                nc.vector.reduce_sum(sum_dst, comp1, axis=AXX)
            sq = evp.tile([P, D], fp16, tag="sq")
            sq_v = sq[:].rearrange("p (h d) -> p h d", h=H)
            nc.vector.tensor_mul(sq_v, comp1, comp1)
            with nc.allow_low_precision(reason="head sumsq fits fp16"):
                nc.vector.reduce_sum(sq_dst, sq_v, axis=AXX)
            invn = sp.tile([P, H], f32, tag="invn")
            nc.scalar.activation(invn[:], sq_dst, ACTF.Abs_reciprocal_sqrt)
            nc.vector.tensor_tensor(
                uk0_dst, comp1,
                invn[:].unsqueeze(2).broadcast_to((P, H, HD)), op=ALU.mult)
            # kvar = (sumsq - sum^2/64)/63
            t1 = sp.tile([P, H], f32, tag="t1")
            nc.vector.tensor_mul(t1[:], sum_dst, sum_dst)
            nc.vector.scalar_tensor_tensor(t1[:], t1[:], -1.0 / HD, sq_dst,
                                           op0=ALU.mult, op1=ALU.add)
            nc.vector.tensor_scalar_mul(kvar_dst, t1[:], 1.0 / (HD - 1))
            # mean for centering; center comp1 in place LAST
            avg = sp.tile([P, H], fp16, tag="avg2")
            nc.vector.tensor_scalar_mul(avg[:], sum_dst, 1.0 / HD)
            nc.vector.tensor_tensor(
                comp1, comp1,
                avg[:].unsqueeze(2).broadcast_to((P, H, HD)), op=ALU.subtract)

        psm = psM.tile([P, 512], f32, tag="pm")
        psm3 = psA.tile([P, 512], f32, tag="po", name="m3")

        def m_chain(t):
            for h in range(H):
                nc.tensor.matmul(
                    psm[:, h * HD:(h + 1) * HD],
                    uk_all[:, t, h, :, :],
                    fv_all[:, t, h * HD:(h + 1) * HD],
                    start=(t == 0 and h == 0), stop=(t == KT - 1))
            nc.tensor.matmul(psm3[0:H, :], kvcol[:, t, :], fv_all[:, t, :],
                             start=(t == 0), stop=(t == KT - 1))

        # ---- k/v pipeline ----
        for t in range(KT):
            xv_t = stage1(xv_all, t, st2_v)
            xk_t = stage1(xk_all, t, st2_k)
            inv_sv = sp.tile([P, 1], f32, tag="invs")
            nc.scalar.activation(inv_sv[:], st2_v[:, t, 1:2],
                                 ACTF.Abs_reciprocal_sqrt, bias=eps_b[:])
            inv_sk = sp.tile([P, 1], f32, tag="invs")
            nc.scalar.activation(inv_sk[:], st2_k[:, t, 1:2],
                                 ACTF.Abs_reciprocal_sqrt, bias=eps_b[:])
            stage2(xv_t, fv_all[:, t, :], False, inv_sv[:])
            comp1 = uk_all[:, t, :, 1, :]
            stage2(xk_t, comp1, True, inv_sk[:])
            kavg = sp.tile([P, H], fp16, tag="avg")
            ksq = sp.tile([P, H], fp16, tag="hsq")
            head_stats(comp1, kavg[:], ksq[:], kvcol[:, t, :],
                       uk_all[:, t, :, 0, :])
            if t >= 2:
                m_chain(t - 2)
        m_chain(KT - 2)
        m_chain(KT - 1)

        # ---- B, RW on the partial sums; AllReduce carries finished values ----
        B_part = cp.tile([P, 512], fp16)
        nc.scalar.activation(B_part[0:HD, :], psm[0:HD, :], ACTF.Copy, scale=cos_w)
        nc.scalar.activation(B_part[HD:P, :], psm[HD:P, :], ACTF.Copy, scale=c_cov)
        R_part = cp.tile([H, 512], fp16)
        nc.vector.scalar_tensor_tensor(R_part[:], psm3[0:H, :], c_var, bdmask[:],
                                       op0=ALU.mult, op1=ALU.mult)
        RT_sb = cp.tile([P, 4, H], fp16)
        for c in range(4):
            pt = psT.tile([P, P], fp16, tag="ptx")
            nc.tensor.transpose(pt[0:P, 0:H], R_part[:, c * P:(c + 1) * P],
                                ident16[0:H, 0:H])
            nc.scalar.copy(RT_sb[:, c, :], pt[0:P, 0:H])
        psrw = psA.tile([P, 512], f32, tag="po", name="rw")
        for c in range(4):
            nc.tensor.matmul(psrw[0:H, :], RT_sb[:, c, :], woT_sb[:, c, :],
                             start=(c == 0), stop=(c == 3))
        RW_part = cp.tile([H, 512], fp16)
        nc.scalar.copy(RW_part[:], psrw[0:H, :])

        cc_in = nc.dram_tensor("cc_in", [P + H, 512], fp16)
        cc_out = nc.dram_tensor("cc_out", [P + H, 512], fp16)
        nc.sync.dma_start(cc_in[0:P, :], B_part[:])
        nc.sync.dma_start(cc_in[P:P + H, :], RW_part[:])
        nc.gpsimd.collective_compute(
            "AllReduce", ALU.add,
            ins=[cc_in[:]], outs=[cc_out[:]],
            replica_groups=[[0, 1], [2, 3], [4, 5], [6, 7]])
        B_sb = cp.tile([P, 512], fp16)
        nc.sync.dma_start(B_sb[:], cc_out[0:P, :])
        RW_sb = cp.tile([H, 512], fp16)
        nc.sync.dma_start(RW_sb[:], cc_out[P:P + H, :])

        # ---- entire q pipeline fills the collective's latency window ----
        for t in range(QT):
            xq_t = stage1(xq_all, t, st2_q)
            inv_sq = sp.tile([P, 1], f32, tag="invs")
            nc.scalar.activation(inv_sq[:], st2_q[:, t, 1:2],
                                 ACTF.Abs_reciprocal_sqrt, bias=eps_b[:])
            comp1 = uq_all[:, t, :, 1, :]
            stage2(xq_t, comp1, True, inv_sq[:])
            qsum = sp.tile([P, H], fp16, tag="avg")
            qsq = sp.tile([P, H], fp16, tag="hsq")
            with nc.allow_low_precision(reason="head sums fit fp16"):
                nc.vector.reduce_sum(qsum[:], comp1, axis=AXX)
            sq = evp.tile([P, D], fp16, tag="sq")
            sq_v = sq[:].rearrange("p (h d) -> p h d", h=H)
            nc.vector.tensor_mul(sq_v, comp1, comp1)
            with nc.allow_low_precision(reason="head sumsq fits fp16"):
                nc.vector.reduce_sum(qsq[:], sq_v, axis=AXX)
            invn = sp.tile([P, H], f32, tag="invn")
            nc.scalar.activation(invn[:], qsq[:], ACTF.Abs_reciprocal_sqrt)
            nc.vector.tensor_tensor(
                uq_all[:, t, :, 0, :], comp1,
                invn[:].unsqueeze(2).broadcast_to((P, H, HD)), op=ALU.mult)
            t1 = sp.tile([P, H], f32, tag="t1")
            nc.vector.tensor_mul(t1[:], qsum[:], qsum[:])
            nc.vector.scalar_tensor_tensor(t1[:], t1[:], -1.0 / HD, qsq[:],
                                           op0=ALU.mult, op1=ALU.add)
            nc.vector.tensor_scalar_mul(qv_all[:, t, :], t1[:], 1.0 / (HD - 1))
            nc.scalar.dma_start_transpose(
                uqT_all[:, t, :, :],
                uq_all[:, t, :, :, :].rearrange("p h two d -> p (h two d)"))
            pq = psT.tile([P, P], fp16, tag="ptx", name=f"pq{t}")
            nc.tensor.transpose(pq[0:H, :], qv_all[:, t, :], ident16[:])
            nc.scalar.copy(qvT_all[:, t, :], pq[0:H, :])

        # ---- attention: per (head, 4-tile quad) batched matmuls ----
        catT_all = cp.tile([P, 4, 512], fp16)
        for q in range(2):
            for hp in range(4):
                psa = psF.tile([P, 512], f32, tag="pf", name=f"psa{q}_{hp}")
                for j in range(2):
                    h = 2 * hp + j
                    nc.tensor.matmul(
                        psa[64 * j:64 * j + 64, :],
                        B_sb[:, h * HD:(h + 1) * HD],
                        uqT_all[:, 4 * q:4 * q + 4, h, :],
                        start=True, stop=True)
                if hp % 2 == 0:
                    nc.scalar.copy(catT_all[:, hp, :], psa[:])
                else:
                    nc.vector.tensor_copy(catT_all[:, hp, :], psa[:])
            for tr in range(4):
                t = 4 * q + tr
                pso = psA.tile([P, D], f32, tag="po")
                for c in range(4):
                    nc.tensor.matmul(
                        pso[:], catT_all[:, c, tr * P:(tr + 1) * P],
                        woT_sb[:, c, :], start=(c == 0), stop=False)
                nc.tensor.matmul(pso[:], qvT_all[:, t, :], RW_sb[:],
                                 start=False, stop=True)
                o_sb = evp.tile([P, D], fp16, tag="o_sb")
                if tr % 2 == 0:
                    nc.vector.tensor_copy(o_sb[:], pso[:])
                else:
                    nc.scalar.copy(o_sb[:], pso[:])
                nc.sync.dma_start(out_d[t * P:(t + 1) * P, :], o_sb[:])

    nc.compile()
    return nc


_NC_CACHE = {}


def kernel(q, k, v, ln_gamma, ln_beta, w_in, w_out, b_out, cov_w_raw, var_w_raw):
    q = np.asarray(q, dtype=np.float32)
    k = np.asarray(k, dtype=np.float32)
    v = np.asarray(v, dtype=np.float32)
    ln_gamma = np.asarray(ln_gamma, dtype=np.float32)
    ln_beta = np.asarray(ln_beta, dtype=np.float32)
    w_in = np.asarray(w_in, dtype=np.float32)
    w_out = np.asarray(w_out, dtype=np.float32)
    b_out = np.asarray(b_out, dtype=np.float32)
    assert np.all(ln_beta == 0.0), "kernel assumes LayerNorm beta == 0"
    assert np.all(b_out == 0.0), "kernel assumes b_out == 0"

    def sigmoid(x):
        return 1.0 / (1.0 + np.exp(-float(x)))

    cov_w = sigmoid(cov_w_raw)
    var_w = sigmoid(var_w_raw)
    cos_w = 1.0 - cov_w - var_w

    wg = w_in * ln_gamma[None, :]
    wgT = np.ascontiguousarray(wg.T).astype(np.float16)
    woT = np.ascontiguousarray(-w_out.T).astype(np.float16)

    q16 = q.astype(np.float16)
    k16 = k.astype(np.float16)
    v16 = v.astype(np.float16)

    key = (round(float(cos_w), 8), round(float(cov_w), 8), round(float(var_w), 8))
    if key not in _NC_CACHE:
        _NC_CACHE[key] = build_kernel(cos_w, cov_w, var_w)
    nc = _NC_CACHE[key]

    in_maps = []
    for c in range(NCORES):
        g, s = c // 2, c % 2
        in_maps.append({
            "xq": np.ascontiguousarray(q16[g, s * TQ:(s + 1) * TQ, :]),
            "xk": np.ascontiguousarray(k16[g, s * TK:(s + 1) * TK, :]),
            "xv": np.ascontiguousarray(v16[g, s * TK:(s + 1) * TK, :]),
            "wgT": wgT,
            "woT": woT,
        })
    res = run_bass_kernel_spmd(nc, in_maps, core_ids=list(range(NCORES))).results

    out = np.empty((QG, N, D), dtype=np.float32)
    for c in range(NCORES):
        g, s = c // 2, c % 2
        out[g, s * TQ:(s + 1) * TQ, :] = res[c]["out"].astype(np.float32)
    return out


# revision 11
# speedup vs baseline: 1.2159x; 1.2159x over previous
"""Trainium2 Bass kernel for nn_Attention_69801808495308.

Softmax-free attention: attn = cos_w*cossim + cov_w*cov/d + var_w*varprod/d is
linear in k-side summaries, so attn @ f_v reassociates into per-head 64x64
matrices (linear-attention trick) - no NxN score matrix is materialized.

Per (group g, head h), with fk/fv/fq the TRUE projected features (the
1/sigma LayerNorm scale is folded into each PSUM evacuation):
  M1 = (fk/||fk||)^T fv        [64,64]
  M2 = (fk - mean(fk))^T fv    [64,64]   (columns sum to 0 -> q-centering free)
  m3 = kvar^T fv               [64]
  out = [cos_w*(fq/||fq||)@M1 + (cov_w/d)*fq@M2] @ woT + qvar @ RW,
        RW = (var_w/d)*blockdiag(m3) @ woT

Sharding: 8 cores = (group g) x (row half s); one pair-wise fp16 AllReduce
(~140KB) of [B; RW] is the only cross-core communication, issued before the
entire q-side pipeline so its latency is hidden.

v3 engine balance (trace-driven):
- two HWDGE rings: ACT ring carries the 6 big fp16 x loads + uqT transposes,
  SP ring carries weights, v-slab transposes, collective I/O and out writes.
- PE warm-up burst keeps the HAM clock-gate at 8/8 from the first real MM.
- fq/fk evacuations write scaled TRUE values straight into the U-tensor
  component-1 slots; per-head means on GpSimd pool; squares/sums split
  between DVE and ACT; per-tile kvar/qvar chains; M-chain + m3 run two
  tiles behind the projections so PE never waits on DVE.
- attention matmuls batched per (head, 4-tile quad); fp16 output.
"""
import numpy as np
from contextlib import ExitStack

import concourse.bass as bass
from concourse import bacc
import concourse.tile as tile
import concourse.mybir as mybir
from concourse.bass_utils import run_bass_kernel_spmd
from concourse.masks import make_identity

f32 = mybir.dt.float32
fp16 = mybir.dt.float16
ALU = mybir.AluOpType
ACTF = mybir.ActivationFunctionType
AXX = mybir.AxisListType.X
POOL_AVG = mybir.PoolFunctionType.avg

QG, N, D = 4, 2048, 512
H, HD = 8, 64
P = 128
LN_EPS = 1e-5
TQ, TK = N // 2, N // 2
QT, KT = TQ // P, TK // P
NCORES = 8


def build_kernel(cos_w, cov_w, var_w):
    c_cov = cov_w / HD
    c_var = var_w / HD

    nc = bacc.Bacc("TRN2", target_bir_lowering=False, debug=False,
                   num_devices=NCORES)
    xq_d = nc.declare_dram_parameter("xq", [TQ, D], fp16, isOutput=False)
    xk_d = nc.declare_dram_parameter("xk", [TK, D], fp16, isOutput=False)
    xv_d = nc.declare_dram_parameter("xv", [TK, D], fp16, isOutput=False)
    wgT_d = nc.declare_dram_parameter("wgT", [D, D], fp16, isOutput=False)
    woT_d = nc.declare_dram_parameter("woT", [D, D], fp16, isOutput=False)
    out_d = nc.declare_dram_parameter("out", [TQ, D], fp16, isOutput=True)

    with tile.TileContext(nc) as tc, ExitStack() as ctx:
        cp = ctx.enter_context(tc.tile_pool(name="cp", bufs=1))
        slp = ctx.enter_context(tc.tile_pool(name="slp", bufs=4))
        sp = ctx.enter_context(tc.tile_pool(name="sp", bufs=8))
        evp = ctx.enter_context(tc.tile_pool(name="evp", bufs=3))

        ident16 = cp.tile([P, P], fp16)
        eps_b = cp.tile([P, 1], f32)
        nc.vector.memset(eps_b[:], LN_EPS)

        xk_all = cp.tile([P, KT, D], fp16)
        xv_all = cp.tile([P, KT, D], fp16)
        xq_all = cp.tile([P, QT, D], fp16)
        wgT_sb = cp.tile([P, 4, D], fp16)
        woT_sb = cp.tile([P, 4, D], fp16)
        fv_all = cp.tile([P, KT, D], fp16)
        uk_all = cp.tile([P, KT, H, 2, HD], fp16)   # [.,.,.,0]=khat [.,.,.,1]=fk
        uq_all = cp.tile([P, QT, H, 2, HD], fp16)   # [.,.,.,0]=qhat [.,.,.,1]=fq
        uqT_all = cp.tile([P, QT, H, P], fp16)
        st2_k = cp.tile([P, KT, 2], f32)
        st2_v = cp.tile([P, KT, 2], f32)
        st2_q = cp.tile([P, QT, 2], f32)
        kvcol = cp.tile([P, KT, H], fp16)
        qv_all = cp.tile([P, QT, H], fp16)
        qvT_all = cp.tile([H, QT, P], fp16)

        # ---- big fp16 loads on the ACT HWDGE ring ----
        nc.scalar.dma_start(
            xk_all[:, 0:KT // 2, :],
            xk_d[0:TK // 2, :].rearrange("(t p) d -> p t d", p=P))
        nc.scalar.dma_start(
            xv_all[:, 0:KT // 2, :],
            xv_d[0:TK // 2, :].rearrange("(t p) d -> p t d", p=P))
        nc.scalar.dma_start(
            xk_all[:, KT // 2:KT, :],
            xk_d[TK // 2:TK, :].rearrange("(t p) d -> p t d", p=P))
        nc.scalar.dma_start(
            xv_all[:, KT // 2:KT, :],
            xv_d[TK // 2:TK, :].rearrange("(t p) d -> p t d", p=P))
        nc.scalar.dma_start(
            xq_all[:, 0:QT // 2, :],
            xq_d[0:TQ // 2, :].rearrange("(t p) d -> p t d", p=P))
        nc.scalar.dma_start(
            xq_all[:, QT // 2:QT, :],
            xq_d[TQ // 2:TQ, :].rearrange("(t p) d -> p t d", p=P))
        # ---- weights on the SP ring ----
        nc.sync.dma_start(wgT_sb[:], wgT_d[:].rearrange("(c p) n -> p c n", p=P))
        nc.sync.dma_start(woT_sb[:], woT_d[:].rearrange("(c p) n -> p c n", p=P))

        make_identity(nc, ident16)
        bdmask = cp.tile([H, 512], f32)
        nc.gpsimd.memset(bdmask[:], 0.0)
        nc.gpsimd.affine_select(
            out=bdmask[:].rearrange("p (b d) -> p b d", b=H),
            in_=bdmask[:].rearrange("p (b d) -> p b d", b=H),
            compare_op=ALU.not_equal, fill=1.0, base=0,
            pattern=[[-1, H], [0, HD]], channel_multiplier=1)

        psF = ctx.enter_context(tc.tile_pool(name="psF", bufs=3, space="PSUM"))
        psT = ctx.enter_context(tc.tile_pool(name="psT", bufs=2, space="PSUM"))
        psM = ctx.enter_context(tc.tile_pool(name="psM", bufs=1, space="PSUM"))
        psA = ctx.enter_context(tc.tile_pool(name="psA", bufs=2, space="PSUM"))

        # ---- PE warm-up: HAM reaches 8/8 while the first loads land ----
        ps_w = psF.tile([P, D], f32, tag="pf", name="warm")
        for i in range(40):
            nc.tensor.matmul(ps_w[:, 0:P], ident16[:], ident16[:],
                             start=True, stop=True)

        def stage1(x_all, t, st2_all):
            xt = x_all[:, t, :]
            st6 = sp.tile([P, 6], f32, tag="st6")
            nc.vector.bn_stats(st6[:], xt)
            nc.vector.bn_aggr(st2_all[:, t, :], st6[:])
            nc.scalar.activation(xt, xt, ACTF.Identity,
                                 bias=st2_all[:, t, 0:1], scale=-1.0)
            return xt

        def stage2(xt, dst, pe_transpose, evac_scale):
            """Transpose (PE or DMA) + 4-matmul projection + scaled ACT evac
            into dst (the TRUE feature values land there)."""
            slab = slp.tile([P, 4, P], fp16, tag="slab")
            if pe_transpose:
                for c in range(4):
                    pt = psT.tile([P, P], fp16, tag="ptx")
                    nc.tensor.transpose(pt[:], xt[:, c * P:(c + 1) * P], ident16[:])
                    if c % 2 == 0:
                        nc.scalar.copy(slab[:, c, :], pt[:])
                    else:
                        nc.vector.tensor_copy(slab[:, c, :], pt[:])
            else:
                nc.sync.dma_start_transpose(slab[:], xt)
            psf = psF.tile([P, D], f32, tag="pf")
            for c in range(4):
                nc.tensor.matmul(psf[:], slab[:, c, :], wgT_sb[:, c, :],
                                 start=(c == 0), stop=(c == 3))
            nc.scalar.activation(dst, psf[:], ACTF.Copy, scale=evac_scale)

        def head_stats(comp1, sum_dst, sq_dst, kvar_dst, uk0_dst):
            """comp1 holds TRUE f [P,H,64]. Per-head sum + sum-sq on DVE,
            khat & kvar derivations; center comp1 in place."""
            with nc.allow_low_precision(reason="head sums fit fp16"):
                nc.vector.reduce_sum(sum_dst, comp1, axis=AXX)
            sq = evp.tile([P, D], fp16, tag="sq")
            sq_v = sq[:].rearrange("p (h d) -> p h d", h=H)
            nc.vector.tensor_mul(sq_v, comp1, comp1)
            with nc.allow_low_precision(reason="head sumsq fits fp16"):
                nc.vector.reduce_sum(sq_dst, sq_v, axis=AXX)
            invn = sp.tile([P, H], f32, tag="invn")
            nc.scalar.activation(invn[:], sq_dst, ACTF.Abs_reciprocal_sqrt)
            nc.vector.tensor_tensor(
                uk0_dst, comp1,
                invn[:].unsqueeze(2).broadcast_to((P, H, HD)), op=ALU.mult)
            # kvar = (sumsq - sum^2/64)/63
            t1 = sp.tile([P, H], f32, tag="t1")
            nc.vector.tensor_mul(t1[:], sum_dst, sum_dst)
            nc.vector.scalar_tensor_tensor(t1[:], t1[:], -1.0 / HD, sq_dst,
                                           op0=ALU.mult, op1=ALU.add)
            nc.vector.tensor_scalar_mul(kvar_dst, t1[:], 1.0 / (HD - 1))
            # mean for centering; center comp1 in place LAST
            avg = sp.tile([P, H], fp16, tag="avg2")
            nc.vector.tensor_scalar_mul(avg[:], sum_dst, 1.0 / HD)
            nc.vector.tensor_tensor(
                comp1, comp1,
                avg[:].unsqueeze(2).broadcast_to((P, H, HD)), op=ALU.subtract)

        psm = psM.tile([P, 512], f32, tag="pm")
        psm3 = psA.tile([P, 512], f32, tag="po", name="m3")

        def m_chain(t):
            for h in range(H):
                nc.tensor.matmul(
                    psm[:, h * HD:(h + 1) * HD],
                    uk_all[:, t, h, :, :],
                    fv_all[:, t, h * HD:(h + 1) * HD],
                    start=(t == 0 and h == 0), stop=(t == KT - 1))
            nc.tensor.matmul(psm3[0:H, :], kvcol[:, t, :], fv_all[:, t, :],
                             start=(t == 0), stop=(t == KT - 1))

        # ---- k/v pipeline ----
        for t in range(KT):
            xv_t = stage1(xv_all, t, st2_v)
            xk_t = stage1(xk_all, t, st2_k)
            inv_sv = sp.tile([P, 1], f32, tag="invs")
            nc.scalar.activation(inv_sv[:], st2_v[:, t, 1:2],
                                 ACTF.Abs_reciprocal_sqrt, bias=eps_b[:])
            inv_sk = sp.tile([P, 1], f32, tag="invs")
            nc.scalar.activation(inv_sk[:], st2_k[:, t, 1:2],
                                 ACTF.Abs_reciprocal_sqrt, bias=eps_b[:])
            stage2(xv_t, fv_all[:, t, :], False, inv_sv[:])
            comp1 = uk_all[:, t, :, 1, :]
            stage2(xk_t, comp1, True, inv_sk[:])
            kavg = sp.tile([P, H], fp16, tag="avg")
            ksq = sp.tile([P, H], fp16, tag="hsq")
            head_stats(comp1, kavg[:], ksq[:], kvcol[:, t, :],
                       uk_all[:, t, :, 0, :])
            if t >= 2:
                m_chain(t - 2)
        m_chain(KT - 2)
        m_chain(KT - 1)

        # ---- B, RW on the partial sums; AllReduce carries finished values ----
        B_part = cp.tile([P, 512], fp16)
        nc.scalar.activation(B_part[0:HD, :], psm[0:HD, :], ACTF.Copy, scale=cos_w)
        nc.scalar.activation(B_part[HD:P, :], psm[HD:P, :], ACTF.Copy, scale=c_cov)
        R_part = cp.tile([H, 512], fp16)
        nc.vector.scalar_tensor_tensor(R_part[:], psm3[0:H, :], c_var, bdmask[:],
                                       op0=ALU.mult, op1=ALU.mult)
        RT_sb = cp.tile([P, 4, H], fp16)
        for c in range(4):
            pt = psT.tile([P, P], fp16, tag="ptx")
            nc.tensor.transpose(pt[0:P, 0:H], R_part[:, c * P:(c + 1) * P],
                                ident16[0:H, 0:H])
            nc.scalar.copy(RT_sb[:, c, :], pt[0:P, 0:H])
        psrw = psA.tile([P, 512], f32, tag="po", name="rw")
        for c in range(4):
            nc.tensor.matmul(psrw[0:H, :], RT_sb[:, c, :], woT_sb[:, c, :],
                             start=(c == 0), stop=(c == 3))
        RW_part = cp.tile([H, 512], fp16)
        nc.scalar.copy(RW_part[:], psrw[0:H, :])

        cc_in = nc.dram_tensor("cc_in", [P + H, 512], fp16)
        cc_out = nc.dram_tensor("cc_out", [P + H, 512], fp16)
        nc.sync.dma_start(cc_in[0:P, :], B_part[:])
        nc.sync.dma_start(cc_in[P:P + H, :], RW_part[:])
        nc.gpsimd.collective_compute(
            "AllReduce", ALU.add,
            ins=[cc_in[:]], outs=[cc_out[:]],
            replica_groups=[[0, 1], [2, 3], [4, 5], [6, 7]])

        # ---- entire q pipeline fills the collective's latency window ----
        for t in range(QT):
            xq_t = stage1(xq_all, t, st2_q)
            inv_sq = sp.tile([P, 1], f32, tag="invs")
            nc.scalar.activation(inv_sq[:], st2_q[:, t, 1:2],
                                 ACTF.Abs_reciprocal_sqrt, bias=eps_b[:])
            comp1 = uq_all[:, t, :, 1, :]
            stage2(xq_t, comp1, True, inv_sq[:])
            qsum = sp.tile([P, H], fp16, tag="avg")
            qsq = sp.tile([P, H], fp16, tag="hsq")
            with nc.allow_low_precision(reason="head sums fit fp16"):
                nc.vector.reduce_sum(qsum[:], comp1, axis=AXX)
            sq = evp.tile([P, D], fp16, tag="sq")
            sq_v = sq[:].rearrange("p (h d) -> p h d", h=H)
            nc.vector.tensor_mul(sq_v, comp1, comp1)
            with nc.allow_low_precision(reason="head sumsq fits fp16"):
                nc.vector.reduce_sum(qsq[:], sq_v, axis=AXX)
            invn = sp.tile([P, H], f32, tag="invn")
            nc.scalar.activation(invn[:], qsq[:], ACTF.Abs_reciprocal_sqrt)
            nc.vector.tensor_tensor(
                uq_all[:, t, :, 0, :], comp1,
                invn[:].unsqueeze(2).broadcast_to((P, H, HD)), op=ALU.mult)
            t1 = sp.tile([P, H], f32, tag="t1")
            nc.vector.tensor_mul(t1[:], qsum[:], qsum[:])
            nc.vector.scalar_tensor_tensor(t1[:], t1[:], -1.0 / HD, qsq[:],
                                           op0=ALU.mult, op1=ALU.add)
            nc.vector.tensor_scalar_mul(qv_all[:, t, :], t1[:], 1.0 / (HD - 1))
            nc.sync.dma_start_transpose(
                uqT_all[:, t, :, :],
                uq_all[:, t, :, :, :].rearrange("p h two d -> p (h two d)"))
            pq = psT.tile([P, P], fp16, tag="ptx", name=f"pq{t}")
            nc.tensor.transpose(pq[0:H, :], qv_all[:, t, :], ident16[:])
            nc.scalar.copy(qvT_all[:, t, :], pq[0:H, :])

        B_sb = cp.tile([P, 512], fp16)
        nc.sync.dma_start(B_sb[:], cc_out[0:P, :])
        RW_sb = cp.tile([H, 512], fp16)
        nc.sync.dma_start(RW_sb[:], cc_out[P:P + H, :])
        # ---- attention: per (head, 4-tile quad) batched matmuls ----
        catT_all = cp.tile([P, 2, 4, 512], fp16)
        for q in range(2):
            for hp in range(4):
                psa = psF.tile([P, 512], f32, tag="pf", name=f"psa{q}_{hp}")
                for j in range(2):
                    h = 2 * hp + j
                    nc.tensor.matmul(
                        psa[64 * j:64 * j + 64, :],
                        B_sb[:, h * HD:(h + 1) * HD],
                        uqT_all[:, 4 * q:4 * q + 4, h, :],
                        start=True, stop=True)
                if hp % 2 == 0:
                    nc.scalar.copy(catT_all[:, q, hp, :], psa[:])
                else:
                    nc.vector.tensor_copy(catT_all[:, q, hp, :], psa[:])
            for tr in range(4):
                t = 4 * q + tr
                pso = psA.tile([P, D], f32, tag="po")
                for c in range(4):
                    nc.tensor.matmul(
                        pso[:], catT_all[:, q, c, tr * P:(tr + 1) * P],
                        woT_sb[:, c, :], start=(c == 0), stop=False)
                nc.tensor.matmul(pso[:], qvT_all[:, t, :], RW_sb[:],
                                 start=False, stop=True)
                o_sb = evp.tile([P, D], fp16, tag="o_sb")
                if tr % 2 == 0:
                    nc.vector.tensor_copy(o_sb[:], pso[:])
                else:
                    nc.scalar.copy(o_sb[:], pso[:])
                nc.sync.dma_start(out_d[t * P:(t + 1) * P, :], o_sb[:])

    nc.compile()
    return nc


_NC_CACHE = {}


def kernel(q, k, v, ln_gamma, ln_beta, w_in, w_out, b_out, cov_w_raw, var_w_raw):
    q = np.asarray(q, dtype=np.float32)
    k = np.asarray(k, dtype=np.float32)
    v = np.asarray(v, dtype=np.float32)
    ln_gamma = np.asarray(ln_gamma, dtype=np.float32)
    ln_beta = np.asarray(ln_beta, dtype=np.float32)
    w_in = np.asarray(w_in, dtype=np.float32)
    w_out = np.asarray(w_out, dtype=np.float32)
    b_out = np.asarray(b_out, dtype=np.float32)
    assert np.all(ln_beta == 0.0), "kernel assumes LayerNorm beta == 0"
    assert np.all(b_out == 0.0), "kernel assumes b_out == 0"

    def sigmoid(x):
        return 1.0 / (1.0 + np.exp(-float(x)))

    cov_w = sigmoid(cov_w_raw)
    var_w = sigmoid(var_w_raw)
    cos_w = 1.0 - cov_w - var_w

    wg = w_in * ln_gamma[None, :]
    wgT = np.ascontiguousarray(wg.T).astype(np.float16)
    woT = np.ascontiguousarray(-w_out.T).astype(np.float16)

    q16 = q.astype(np.float16)
    k16 = k.astype(np.float16)
    v16 = v.astype(np.float16)

    key = (round(float(cos_w), 8), round(float(cov_w), 8), round(float(var_w), 8))
    if key not in _NC_CACHE:
        _NC_CACHE[key] = build_kernel(cos_w, cov_w, var_w)
    nc = _NC_CACHE[key]

    in_maps = []
    for c in range(NCORES):
        g, s = c // 2, c % 2
        in_maps.append({
            "xq": np.ascontiguousarray(q16[g, s * TQ:(s + 1) * TQ, :]),
            "xk": np.ascontiguousarray(k16[g, s * TK:(s + 1) * TK, :]),
            "xv": np.ascontiguousarray(v16[g, s * TK:(s + 1) * TK, :]),
            "wgT": wgT,
            "woT": woT,
        })
    res = run_bass_kernel_spmd(nc, in_maps, core_ids=list(range(NCORES))).results

    out = np.empty((QG, N, D), dtype=np.float32)
    for c in range(NCORES):
        g, s = c // 2, c % 2
        out[g, s * TQ:(s + 1) * TQ, :] = res[c]["out"].astype(np.float32)
    return out
